# revision 21
# baseline (speedup 1.0000x reference)
"""Multi-head attention (B=2, S=4096, E=768, H=12, dk=64) on 8 Trainium2 NeuronCores.

Sharding: the 24 (batch, head) pairs are split across the 8 cores -> each core
computes full-sequence attention for 3 heads of one batch (tensor parallel over
heads, data parallel over batch, per the sharding hint).  The output projection
is resharded via an in-kernel AllGather of the (normalized) context within each
4-core batch group; each core then applies its own 192-column slice of Wf, so it
emits a disjoint column block of the output and the host-side unshard is a pure
concatenation.

Per-core dataflow (bf16 compute, fp32 PSUM accumulation):
  x -> bf16 (SWDGE cast DMA) -> x^T (DMA xbar transpose)
  qT_h / kT_h = (Wh | Wh)^T x^T     duplicated over both partition halves so the
                                    scores matmuls can be row-packed two-at-a-time
                                    (dk=64 contraction only fills half the PE array)
  v_h = x Wv_h (+ ones column)      natural layout; the ones column makes the
                                    context matmul also emit the softmax denominator
  scoresT[k, q] = kT^T qT           (PE, transposed so softmax sums ride the PE)
  expT = exp(scores / sqrt(dk))     (ACT, PSUM->SBUF, scale fused)
  ctxT | rowsum = [v|1]^T expT      (PE, accumulated over k tiles)
  ctx_norm = ctxT * (1/rowsum)      (DVE, with a PE-broadcast reciprocal)
  AllGather(ctx_norm)               within the 4-core batch group
  out[:, cols] = ctx_full^T Wf_cols (PE) for this core's 192-column slice of Wf

The mask input is all ones for this problem (spec fill="ones"); a host-side
numpy fallback handles the general masked case.
"""

import math

import numpy as np

B, S, E, HEADS, DK = 2, 4096, 768, 12, 64
NCORES = 8
HPC = 3  # heads per core
GROUP = 4  # cores per batch group
P = 128
SQ = 512  # q-block (columns per scores matmul)
KG = 3  # k-tiles per exp group (3 PSUM banks per scores slab)
GROUPS = [[0, 1, 2, 3], [4, 5, 6, 7]]

_cache = {}


def _build(s=S, use_f32r_bcast=True, collective=True, tmode="dma", chain_order="kqv"):
    import concourse.bass as bass
    import concourse.mybir as mybir
    import concourse.tile as tile
    from concourse import bacc
    from concourse.bass import ts, ds

    dt = mybir.dt
    f32, bf16 = dt.float32, dt.bfloat16
    f32r = dt.float32r if use_f32r_bcast else dt.float32

    KT = s // P  # k tiles
    NQ = s // SQ if s >= SQ else 1
    sq = min(SQ, s)
    ET = E // P  # contraction tiles over E
    SCALE = 1.0 / math.sqrt(DK)
    W2 = 2 * DK  # duplicated head width
    SH = 2 if NQ % 2 == 0 else 1  # s-halves for the load/transpose/proj pipeline
    s2 = s // SH
    NQ2 = NQ // SH
    KT2 = KT // SH
    KG = 2  # k-tiles per exp group (2 PSUM banks per scores slab)

    nc = bacc.Bacc("TRN2", target_bir_lowering=False, debug=False, num_devices=NCORES)

    OC = E // GROUP  # output columns per core (192)
    xin = {
        "q": nc.dram_tensor("xq", [s, E], f32, kind="ExternalInput"),
        "k": nc.dram_tensor("xk", [s, E], f32, kind="ExternalInput"),
        "v": nc.dram_tensor("xv", [s, E], f32, kind="ExternalInput"),
    }
    wq = nc.dram_tensor("wq", [E, W2 * HPC], f32, kind="ExternalInput")
    wk = nc.dram_tensor("wk", [E, W2 * HPC], f32, kind="ExternalInput")
    wv = nc.dram_tensor("wv", [E, DK * HPC], f32, kind="ExternalInput")
    wf = nc.dram_tensor("wf", [E, OC], f32, kind="ExternalInput")
    bq = nc.dram_tensor("bq", [W2 * HPC], f32, kind="ExternalInput")
    bk = nc.dram_tensor("bk", [W2 * HPC], f32, kind="ExternalInput")
    bv = nc.dram_tensor("bv", [DK * HPC], f32, kind="ExternalInput")
    bfin = nc.dram_tensor("bfin", [OC], f32, kind="ExternalInput")
    out = nc.dram_tensor("out", [s, OC], f32, kind="ExternalOutput")

    # DRAM staging
    x_bf = {n: nc.dram_tensor(f"x{n}_bf", [s, E], bf16) for n in ("q", "k", "v")}
    SPL = 4 if NQ % 4 == 0 else (2 if NQ % 2 == 0 else 1)  # collective chunks/head
    sB = s // SPL
    send = [
        [nc.dram_tensor(f"send{h}_{c}", [DK, sB], bf16) for c in range(SPL)]
        for h in range(HPC)
    ]
    recv = [
        [nc.dram_tensor(f"recv{h}_{c}", [GROUP, DK, sB], bf16) for c in range(SPL)]
        for h in range(HPC)
    ]

    mult = mybir.AluOpType.mult
    Exp = mybir.ActivationFunctionType.Exp

    with tile.TileContext(nc) as tc:
        with (
            tc.tile_pool(name="sb", bufs=1) as sb,
            tc.tile_pool(name="ps", bufs=1, space="PSUM") as ps,
        ):
            # ---- weight / bias loads (small, first in the SWDGE queue) ----
            wq_sb = sb.tile([P, ET, W2 * HPC], bf16, tag="wq")
            wk_sb = sb.tile([P, ET, W2 * HPC], bf16, tag="wk")
            wv_sb = sb.tile([P, ET, DK * HPC], bf16, tag="wv")
            wf_sb = sb.tile([P, ET, OC], bf16, tag="wf")
            for t_sb, t_dr in ((wq_sb, wq), (wk_sb, wk), (wv_sb, wv), (wf_sb, wf)):
                nc.gpsimd.dma_start(
                    out=t_sb, in_=t_dr.ap().rearrange("(t p) o -> p t o", p=P)
                )
            bq_sb = sb.tile([P, HPC], f32, tag="bq")
            bk_sb = sb.tile([P, HPC], f32, tag="bk")
            nc.gpsimd.dma_start(out=bq_sb, in_=bq.ap().rearrange("(h p) -> p h", p=P))
            nc.gpsimd.dma_start(out=bk_sb, in_=bk.ap().rearrange("(h p) -> p h", p=P))
            bv_row = sb.tile([1, DK * HPC], bf16, tag="bv_row")
            nc.gpsimd.dma_start(out=bv_row, in_=bv.ap()[None, :])
            bf_row = sb.tile([1, OC], bf16, tag="bf_row")
            nc.gpsimd.dma_start(out=bf_row, in_=bfin.ap()[None, :])

            ones_bf = sb.tile([1, P], bf16, tag="ones_bf")
            nc.vector.memset(ones_bf, 1.0)
            ones_r = sb.tile([1, DK], f32, tag="ones_r")
            nc.vector.memset(ones_r, 1.0)

            # ---- stage dtype casts (DRAM -> DRAM bf16, SWDGE), s-half chunks ----
            CHAIN = [(n, sh) for n in chain_order for sh in range(SH)]
            if tmode == "dma":
                for n, sh in CHAIN:
                    nc.gpsimd.dma_start(
                        out=x_bf[n].ap()[ds(sh * s2, s2), :],
                        in_=xin[n].ap()[ds(sh * s2, s2), :],
                    )
            else:
                from concourse.masks import make_identity

                ident = sb.tile([P, P], bf16, tag="ident")
                make_identity(nc, ident)

            # ---- v' slabs (per k-half, ones column for softmax denominators) ----
            KVH = max(1, KT // 2)
            vs_parts = []
            for half in range((KT + KVH - 1) // KVH):
                nk = min(KVH, KT - half * KVH)
                vst = sb.tile(
                    [P, HPC * nk * (DK + 1)], bf16, tag=f"vs{half}", name=f"vs{half}"
                )
                v4 = vst.rearrange("p (h c x) -> p h c x", h=HPC, x=DK + 1)
                nc.vector.memset(v4[:, :, :, DK : DK + 1], 1.0)
                vs_parts.append(v4)

            def vs4(h, k):
                return vs_parts[k // KVH][:, h, k % KVH, :]

            # ---- transpose + project, pipelined over (s-half, input) ----
            qT = [None] * HPC  # [P, s] bf16, dk duplicated across partition halves
            kT = [None] * HPC
            for h in range(HPC):
                qT[h] = sb.tile([P, s], bf16, tag=f"qT{h}", name=f"qT{h}")
                kT[h] = sb.tile([P, s], bf16, tag=f"kT{h}", name=f"kT{h}")

            for n, sh in CHAIN:
                if True:
                    xT = sb.tile(
                        [P, ET, s2], bf16, tag="xT", bufs=2, name=f"xT{n}{sh}"
                    )
                    if tmode == "dma":
                        for c in range(ET):
                            nc.sync.dma_start(
                                out=xT[:, c, :],
                                in_=x_bf[n].ap()[ds(sh * s2, s2), c * P : (c + 1) * P],
                                transpose=True,
                            )
                    else:
                        # cast-load natural chunks, transpose on the PE
                        NCH = 4  # s-tiles per chunk
                        for ch in range(s2 // (NCH * P)):
                            nat = sb.tile(
                                [P, NCH, E], bf16, tag="nat", bufs=3, name="nat"
                            )
                            nc.gpsimd.dma_start(
                                out=nat,
                                in_=xin[n]
                                .ap()[ds(sh * s2 + ch * NCH * P, NCH * P), :]
                                .rearrange("(t p) e -> p t e", p=P),
                            )
                            for c in range(ET):
                                tp = ps.tile(
                                    [P, NCH * P], bf16, tag="proj", bufs=2, name="tp"
                                )
                                for i in range(NCH):
                                    nc.tensor.transpose(
                                        tp[:, ts(i, P)],
                                        in_=nat[:, i, c * P : (c + 1) * P],
                                        identity=ident,
                                    )
                                nc.vector.tensor_copy(
                                    xT[:, c, ds(ch * NCH * P, NCH * P)], tp
                                )
                    if n in ("q", "k"):
                        w_sb = wq_sb if n == "q" else wk_sb
                        b_sb = bq_sb if n == "q" else bk_sb
                        dst_list = qT if n == "q" else kT
                        for h in range(HPC):
                            for j2 in range(NQ2):
                                j = sh * NQ2 + j2
                                acc = ps.tile(
                                    [P, sq], f32, tag="proj", bufs=2, name="acc"
                                )
                                for t in range(ET):
                                    nc.tensor.matmul(
                                        acc,
                                        lhsT=w_sb[:, t, h * W2 : (h + 1) * W2],
                                        rhs=xT[:, t, ts(j2, sq)],
                                        start=(t == 0),
                                        stop=(t == ET - 1),
                                    )
                                nc.vector.tensor_scalar_add(
                                    dst_list[h][:, ts(j, sq)], acc, b_sb[:, h : h + 1]
                                )
                    else:
                        for sc2 in range(KT2):
                            sc = sh * KT2 + sc2
                            acc = ps.tile(
                                [P, HPC * DK], f32, tag="proj", bufs=2, name="vacc"
                            )
                            for t in range(ET):
                                nc.tensor.matmul(
                                    acc,
                                    lhsT=xT[:, t, ts(sc2, P)],
                                    rhs=wv_sb[:, t, :],
                                    start=(t == 0),
                                    stop=False,
                                )
                            nc.tensor.matmul(
                                acc, lhsT=ones_bf, rhs=bv_row, start=False, stop=True
                            )
                            accv = acc.rearrange("p (h d) -> p h d", h=HPC)
                            for hh in range(HPC):
                                nc.vector.tensor_copy(
                                    vs4(hh, sc)[:, 0:DK], accv[:, hh, :]
                                )

            # ---- attention ----
            KH = max(1, KT // 2)  # k-tiles per expT half-slab
            ctxn = [None] * HPC
            for h in range(HPC):
                ctxn[h] = sb.tile([DK, s], bf16, tag="ctxn", bufs=3, name=f"ctxn{h}")
                for j in range(NQ):
                    halves = []
                    for half in range((KT + KH - 1) // KH):
                        expT = sb.tile(
                            [P, KH * sq], bf16, tag="expT", bufs=2, name="expT"
                        )
                        halves.append(expT)
                        for g0 in range(0, KH, KG):
                            grp = range(g0, min(g0 + KG, KH))
                            sc_ps = ps.tile(
                                [P, len(grp) * sq], f32, tag="big", bufs=2, name="sc"
                            )
                            for i, gk in enumerate(grp):
                                k = half * KH + gk
                                lo = (k % 2) * DK
                                nc.tensor.matmul(
                                    sc_ps[:, ts(i, sq)],
                                    lhsT=kT[h][lo : lo + DK, ts(k, P)],
                                    rhs=qT[h][lo : lo + DK, ts(j, sq)],
                                    start=True,
                                    stop=True,
                                )
                            nc.scalar.activation(
                                out=expT[:, ds(g0 * sq, len(grp) * sq)],
                                in_=sc_ps,
                                func=Exp,
                                scale=SCALE,
                            )
                    ctx = ps.tile([DK + 1, sq], f32, tag="ctx", bufs=2, name="ctx")
                    for k in range(KT):
                        nc.tensor.matmul(
                            ctx,
                            lhsT=vs4(h, k),
                            rhs=halves[k // KH][:, ts(k % KH, sq)],
                            start=(k == 0),
                            stop=(k == KT - 1),
                        )
                    # normalize: ctx[0:DK] / rowsum (rowsum = ctx row DK)
                    recip = sb.tile([1, sq], f32, tag="recip", bufs=1, name="recip")
                    nc.vector.reciprocal(recip, ctx[DK : DK + 1, :])
                    bc = ps.tile([DK, sq], f32, tag="ctx", bufs=2, name="bc")
                    nc.tensor.matmul(bc, lhsT=ones_r, rhs=recip, start=True, stop=True)
                    bc_sb = sb.tile([DK, sq], f32, tag="bc_sb", bufs=1, name="bc_sb")
                    nc.vector.tensor_copy(bc_sb, bc)
                    nc.vector.tensor_tensor(
                        ctxn[h][:, ts(j, sq)], ctx[0:DK, :], bc_sb, mult
                    )
                    # ship finished s-chunks of this head's context (AllGather)
                    if (j + 1) % (NQ // SPL) == 0:
                        c = j // (NQ // SPL)
                        nc.sync.dma_start(
                            out=send[h][c].ap(), in_=ctxn[h][:, ds(c * sB, sB)]
                        )
                        if collective:
                            nc.gpsimd.collective_compute(
                                "AllGather",
                                mybir.AluOpType.bypass,
                                replica_groups=GROUPS,
                                ins=[send[h][c].ap().opt()],
                                outs=[recv[h][c].ap().opt()],
                            )
                        else:  # timeline-sim stand-in at collective-like volume
                            for r in range(GROUP):
                                nc.sync.dma_start(
                                    out=recv[h][c].ap()[r], in_=send[h][c].ap()
                                )

            bfp = bf_row  # v-bias already folded into ctx; out bias is just bf

            # ---- output projection: out[:, cols] = ctx_full^T Wf_cols + bfp ----
            SG = min(512, s)  # s-rows per gathered-context chunk
            for sg in range(s // SG):
                cb = (sg * SG) // sB
                ctxf = sb.tile([P, ET, SG], bf16, tag="ctxf", bufs=2, name="ctxf")
                for r in range(GROUP):
                    for h in range(HPC):
                        g = r * HPC * DK + h * DK
                        nc.sync.dma_start(
                            out=ctxf[g % P : g % P + DK, g // P, :],
                            in_=recv[h][cb].ap()[r][
                                :, ds(sg * SG - cb * sB, SG)
                            ],
                        )
                for c in range(SG // P):
                    po = ps.tile([P, OC], f32, tag="proj", bufs=2, name="po")
                    for t in range(ET):
                        nc.tensor.matmul(
                            po,
                            lhsT=ctxf[:, t, ts(c, P)],
                            rhs=wf_sb[:, t, :],
                            start=(t == 0),
                            stop=False,
                        )
                    nc.tensor.matmul(
                        po, lhsT=ones_bf, rhs=bfp, start=False, stop=True
                    )
                    osb = sb.tile([P, OC], f32, tag="osb", bufs=2, name="osb")
                    nc.vector.tensor_copy(osb, po)
                    nc.sync.dma_start(
                        out=out.ap()[ds(sg * SG + c * P, P), :], in_=osb
                    )

    nc.compile()
    return nc


def shard_inputs(inputs, s=S):
    """Build the 8 per-core input maps from the full problem inputs."""
    q = np.ascontiguousarray(np.asarray(inputs["query"], np.float32))
    k = np.ascontiguousarray(np.asarray(inputs["key"], np.float32))
    v = np.ascontiguousarray(np.asarray(inputs["value"], np.float32))
    Wq = np.asarray(inputs["Wq"], np.float32)
    Wk = np.asarray(inputs["Wk"], np.float32)
    Wv = np.asarray(inputs["Wv"], np.float32)
    Wf = np.asarray(inputs["Wf"], np.float32)
    bqv = np.asarray(inputs["bq"], np.float32)
    bkv = np.asarray(inputs["bk"], np.float32)
    bvv = np.asarray(inputs["bv"], np.float32)
    bfv = np.asarray(inputs["bf"], np.float32)

    maps = []
    for core in range(NCORES):
        b = core // GROUP
        rank = core % GROUP
        heads = [HPC * rank + i for i in range(HPC)]
        wq_dev = np.empty((E, 2 * DK * HPC), np.float32)
        wk_dev = np.empty((E, 2 * DK * HPC), np.float32)
        bq_dev = np.empty((2 * DK * HPC,), np.float32)
        bk_dev = np.empty((2 * DK * HPC,), np.float32)
        wv_dev = np.empty((E, DK * HPC), np.float32)
        bv_dev = np.empty((DK * HPC,), np.float32)
        for i, h in enumerate(heads):
            cols = slice(DK * h, DK * (h + 1))
            wq_dev[:, 128 * i : 128 * i + 64] = Wq[:, cols]
            wq_dev[:, 128 * i + 64 : 128 * i + 128] = Wq[:, cols]
            wk_dev[:, 128 * i : 128 * i + 64] = Wk[:, cols]
            wk_dev[:, 128 * i + 64 : 128 * i + 128] = Wk[:, cols]
            bq_dev[128 * i : 128 * i + 64] = bqv[cols]
            bq_dev[128 * i + 64 : 128 * i + 128] = bqv[cols]
            bk_dev[128 * i : 128 * i + 64] = bkv[cols]
            bk_dev[128 * i + 64 : 128 * i + 128] = bkv[cols]
            wv_dev[:, DK * i : DK * (i + 1)] = Wv[:, cols]
            bv_dev[DK * i : DK * (i + 1)] = bvv[cols]
        ocols = slice((E // GROUP) * rank, (E // GROUP) * (rank + 1))
        maps.append(
            {
                "xq": q[b, :s],
                "xk": k[b, :s],
                "xv": v[b, :s],
                "wq": np.ascontiguousarray(wq_dev),
                "wk": np.ascontiguousarray(wk_dev),
                "wv": np.ascontiguousarray(wv_dev),
                "wf": np.ascontiguousarray(Wf[:, ocols]),
                "bq": bq_dev,
                "bk": bk_dev,
                "bv": bv_dev,
                "bfin": np.ascontiguousarray(bfv[ocols]),
            }
        )
    return maps


def assemble_output(per_core_outs, s=S):
    oc = E // GROUP
    full = np.empty((B, s, E), np.float32)
    for core in range(NCORES):
        b = core // GROUP
        rank = core % GROUP
        full[b, :, oc * rank : oc * (rank + 1)] = per_core_outs[core]
    return full


def _reference_numpy(query, key, value, mask, Wq, bq, Wk, bk, Wv, bv, Wf, bf):
    """Host fallback for the general (masked) case; not used for all-ones masks."""
    Bn, Sn, En = query.shape
    scale = 1.0 / math.sqrt(DK)

    def split(x):
        return x.reshape(Bn, Sn, HEADS, DK).transpose(0, 2, 1, 3)

    q = split(query @ Wq + bq)
    k = split(key @ Wk + bk)
    v = split(value @ Wv + bv)
    scores = np.einsum("bhqd,bhkd->bhqk", q, k) * scale
    scores = np.where(np.asarray(mask)[:, :, :, :] == 0, -1.0e9, scores)
    scores -= scores.max(axis=-1, keepdims=True)
    e = np.exp(scores)
    attn = e / e.sum(axis=-1, keepdims=True)
    ctx = np.einsum("bhqk,bhkd->bhqd", attn, v)
    ctx = ctx.transpose(0, 2, 1, 3).reshape(Bn, Sn, En)
    return (ctx @ Wf + bf).astype(np.float32)


LAST_RESULTS = None


def kernel(**inputs):
    global LAST_RESULTS
    mask = inputs.get("mask")
    if mask is not None and not np.all(np.asarray(mask) != 0):
        return _reference_numpy(
            np.asarray(inputs["query"], np.float32),
            np.asarray(inputs["key"], np.float32),
            np.asarray(inputs["value"], np.float32),
            mask,
            *[np.asarray(inputs[k], np.float32) for k in
              ("Wq", "bq", "Wk", "bk", "Wv", "bv", "Wf", "bf")],
        )

    import sys
    import types

    try:
        import antenv.axon_hooks  # noqa: F401
    except ImportError:
        # This container's axon build lacks the NTFF profile hook; stub it so
        # BASS_TRACE=1 degrades to an untraced run instead of crashing.
        stub = types.ModuleType("antenv.axon_hooks")
        stub.get_axon_ntff_profile_hook = lambda: None
        sys.modules["antenv.axon_hooks"] = stub

    from concourse import bass_utils

    if "nc" not in _cache:
        _cache["nc"] = _build()
    nc = _cache["nc"]
    in_maps = shard_inputs(inputs)
    res = bass_utils.run_bass_kernel_spmd(nc, in_maps, core_ids=list(range(NCORES)))
    LAST_RESULTS = res
    return assemble_output([res.results[c]["out"] for c in range(NCORES)])
